# revision 39
# baseline (speedup 1.0000x reference)
"""Trainium2 Bass kernel for a 6-level db4 DWT (zero-padding mode).

Input x: [64, 262144] f32. Output (cA6, cD6, cD5, cD4, cD3, cD2, cD1).

Per NeuronCore (batch sharded 8 rows/core):
  - fp16 datapath, fp32 PSUM accumulation, fp32 DRAM in/out.
  - Signal kept partition-major in SBUF (col j = a[128j..128j+128) down the
    partitions), built via a cast-on-DMA load + one batched X-bar DMA
    transpose per row (f16, 128x128 blocks).
  - One DWT level = banded fp16 matmuls: per 128-coeff output segment s',
    three accumulating matmuls read input cols 2s'-1 (6-tap halo), 2s',
    2s'+1.  cA lands partition-major (next level's input) via an ACT cast
    copy; cD is staged to fp16, X-bar-transposed to [segment, coeff] layout,
    cast to f32 on DVE and stored with 512B-contiguous DMA chunks.
  - Levels 1-3 per batch row; levels 4-6 pack all rows into one tile.
"""

import sys

sys.path.insert(0, "/opt/trn_rl_repo")

import numpy as np

import concourse.bacc as bacc
import concourse.mybir as mybir
import concourse.tile as tile

F32 = mybir.dt.float32
F16 = mybir.dt.float16

N0 = 262144
B_FULL = 64
N_CORES = 8
ROWS = B_FULL // N_CORES
LEVEL = 6
F = 8
LEAD = 16  # leading zero cols (keeps X-bar dst 32B-aligned; col 15 = halo zero)

DEC_LO = np.array([-0.010597401784997278, 0.032883011666982945, 0.030841381835986965,
                   -0.18703481171888114, -0.02798376941698385, 0.6308807679295904,
                   0.7148465705525415, 0.23037781330885523], dtype=np.float64)
DEC_HI = np.array([-0.23037781330885523, 0.7148465705525415, -0.6308807679295904,
                   -0.02798376941698385, 0.18703481171888114, 0.030841381835986965,
                   -0.032883011666982945, -0.010597401784997278], dtype=np.float64)
W_LO = DEC_LO[::-1]
W_HI = DEC_HI[::-1]


def _ceil(a, b):
    return -(-a // b)


def make_band_arrays():
    """fp16 constants: wmain [128, 512] = [A2|A3|D2|D3], whalo [128, 256] =
    [A1|D1] (data only in rows 122..127)."""
    mats = {}
    for name, w in (("A", W_LO), ("D", W_HI)):
        m1 = np.zeros((128, 128), np.float32)
        m2 = np.zeros((128, 128), np.float32)
        m3 = np.zeros((128, 128), np.float32)
        for c in range(128):
            for j in range(F):
                u = 2 * c + j - 6
                if u < 0:
                    m1[122 + u + 6, c] = w[j]
                elif u < 128:
                    m2[u, c] = w[j]
                else:
                    m3[u - 128, c] = w[j]
        mats[name] = (m1, m2, m3)
    wmain = np.concatenate(
        [mats["A"][1], mats["A"][2], mats["D"][1], mats["D"][2]], axis=1)
    whalo = np.concatenate([mats["A"][0], mats["D"][0]], axis=1)
    return wmain.astype(np.float16), whalo.astype(np.float16)


def plan_levels(n0=N0):
    levels = []
    n = n0
    for _ in range(LEVEL):
        c = _ceil(n, 128)          # data cols of the input signal
        m = (n + F - 1) // 2       # output length
        s = _ceil(m, 128)          # output segments
        sp = s + (s % 2)           # padded to even
        w = LEAD + 2 * sp          # tile width per row (even)
        nb = _ceil(s, 128)         # 128-seg blocks for output staging
        levels.append(dict(n=n, c=c, m=m, s=s, sp=sp, w=w, nb=nb))
        n = m
    return levels


def chunk_sizes(sp, cap=512):
    assert sp % 2 == 0
    half = sp // 2
    nch = max(1, _ceil(half, cap // 2))
    base, rem = divmod(half, nch)
    return [2 * (base + (1 if i < rem else 0)) for i in range(nch)]


def row_chunks(rows, sp, cap=512):
    per = max(1, cap // sp)
    nch = _ceil(rows, per)
    base, rem = divmod(rows, nch)
    return [base + (1 if i < rem else 0) for i in range(nch)]


OUT_NAMES = ["a6", "d6", "d5", "d4", "d3", "d2", "d1"]


def build_nc(rows=ROWS, n0=N0):
    levels = plan_levels(n0)
    nc = bacc.Bacc(None, target_bir_lowering=False)

    x_in = nc.declare_dram_parameter("x", [rows, n0], F32, isOutput=False)
    wm_in = nc.declare_dram_parameter("wmain", [128, 512], F16, isOutput=False)
    wh_in = nc.declare_dram_parameter("whalo", [128, 256], F16, isOutput=False)

    d_out = [nc.declare_dram_parameter(f"d{l + 1}", [rows, levels[l]["m"]], F32,
                                       isOutput=True) for l in range(LEVEL)]
    a6_out = nc.declare_dram_parameter("a6", [rows, levels[-1]["m"]], F32,
                                       isOutput=True)

    with tile.TileContext(nc) as tc:
        with (
            tc.tile_pool(name="consts", bufs=1) as consts,
            tc.tile_pool(name="fpool", bufs=2) as fpool,
            tc.tile_pool(name="inp", bufs=2) as inp,
            tc.tile_pool(name="packed", bufs=1) as packed,
            tc.tile_pool(name="sd", bufs=2) as sdp,
            tc.tile_pool(name="outp", bufs=2) as outp,
            tc.tile_pool(name="pa", bufs=3, space="PSUM") as pa,
            tc.tile_pool(name="pd", bufs=3, space="PSUM") as pd,
        ):
            wm = consts.tile([128, 512], F16, tag="wm")
            wh = consts.tile([128, 256], F16, tag="wh")
            nc.sync.dma_start(out=wm[:], in_=wm_in[:])
            nc.sync.dma_start(out=wh[:], in_=wh_in[:])
            zsrc = consts.tile([128, 256], F16, tag="zsrc")
            nc.gpsimd.memset(zsrc[:], 0.0)

            def zero_cols(dst_ap):
                n = dst_ap.shape[-1]
                nc.vector.tensor_copy(dst_ap, zsrc[:, 0:n])

            lhs = {  # (halo, main, odd) per filter
                "A": (wh[:, 0:128], wm[:, 0:128], wm[:, 128:256]),
                "D": (wh[:, 128:256], wm[:, 256:384], wm[:, 384:512]),
            }

            def conv_chunk(in_pairs, psum_t, filt, s0, ns, extra=None):
                """3 accumulating fp16 matmuls for out segs [s0, s0+ns).
                pair t = tile cols (14+2t, 15+2t); col 2s'-1 -> pair s' idx 1,
                col 2s' -> pair s'+1 idx 0, col 2s'+1 -> pair s'+1 idx 1."""
                m1, m2, m3 = lhs[filt]
                if extra is None:
                    ra = in_pairs[:, s0:s0 + ns, 1]
                    rb = in_pairs[:, s0 + 1:s0 + 1 + ns, 0]
                    rc = in_pairs[:, s0 + 1:s0 + 1 + ns, 1]
                else:
                    r0, nr = extra
                    ra = in_pairs[:, r0:r0 + nr, s0:s0 + ns, 1]
                    rb = in_pairs[:, r0:r0 + nr, s0 + 1:s0 + 1 + ns, 0]
                    rc = in_pairs[:, r0:r0 + nr, s0 + 1:s0 + 1 + ns, 1]
                nc.tensor.matmul(psum_t, m1, ra, start=True, stop=False)
                nc.tensor.matmul(psum_t, m2, rb, start=False, stop=False)
                nc.tensor.matmul(psum_t, m3, rc, start=False, stop=True)

            def pairs_view(cur, rows_packed=None):
                if rows_packed is None:
                    width = cur.shape[-1]
                    return cur[:, 14:width].rearrange("p (s two) -> p s two",
                                                      two=2)
                rv = cur.rearrange("p (r w) -> p r w", r=rows_packed)
                w_ = rv.shape[-1]
                return rv[:, :, 14:w_].rearrange("p r (s two) -> p r s two",
                                                 two=2)

            def emit_out(sd_t, lv, nblk):
                """X-bar transpose fp16 SD -> T, cast to f32 OUT on DVE."""
                t_t = outp.tile([128, 128 * nblk], F16, tag=f"t{nblk}")
                nc.scalar.dma_start(
                    out=t_t.rearrange("p (t f) -> p t f", f=128),
                    in_=sd_t[:, 0:128 * nblk], transpose=True)
                out_t = outp.tile([128, 128 * nblk], F32, tag=f"o{nblk}")
                nc.vector.tensor_copy(out_t[:], t_t[:])
                return out_t

            def dma_out(out_t, dram_t, r, lv, base_col=0):
                m = lv["m"]
                full_segs = m // 128
                nbf = full_segs // 128
                tail = m - 128 * full_segs
                if nbf:
                    dst = dram_t[r, 0:16384 * nbf].rearrange(
                        "(b j c) -> j b c", j=128, c=128)
                    src = out_t[:, base_col:base_col + 128 * nbf].rearrange(
                        "p (b c) -> p b c", c=128)
                    nc.sync.dma_start(out=dst, in_=src)
                rem = full_segs - 128 * nbf
                if rem:
                    dst = dram_t[r, 16384 * nbf:16384 * nbf + 128 * rem].rearrange(
                        "(j c) -> j c", c=128)
                    src = out_t[0:rem, base_col + 128 * nbf:base_col + 128 * (nbf + 1)]
                    nc.sync.dma_start(out=dst, in_=src)
                if tail:
                    p_t = full_segs % 128
                    b_t = full_segs // 128
                    dst = dram_t[r:r + 1, 128 * full_segs:m]
                    src = out_t[p_t:p_t + 1,
                                base_col + 128 * b_t:base_col + 128 * b_t + tail]
                    nc.scalar.dma_start(out=dst, in_=src)

            # ---------- per-row levels 1..3 ----------
            lv4 = levels[3]
            in4 = packed.tile([128, rows * lv4["w"]], F16, tag="in4")
            in4_rows = in4.rearrange("p (r w) -> p r w", r=rows)

            for r in range(rows):
                # free-major cast-load: ftile[p, 128t+f] = x[r, 16384t+128p+f]
                c1 = n0 // 128
                ftile = fpool.tile([128, c1], F16, tag="f")
                src = x_in[r].rearrange("(t p f) -> p t f", p=128, f=128)
                nc.gpsimd.dma_start(
                    out=ftile.rearrange("p (t f) -> p t f", f=128), in_=src)

                lv1 = levels[0]
                in1 = inp.tile([128, lv1["w"]], F16, tag="in1")
                zero_cols(in1[:, 0:LEAD])
                zero_cols(in1[:, LEAD + lv1["c"]:lv1["w"]])
                # one batched X-bar transpose: in1 col LEAD+128t+c = a[128*(128t+c)+p]
                nc.sync.dma_start(
                    out=in1[:, LEAD:LEAD + c1].rearrange("p (t f) -> p t f", f=128),
                    in_=ftile[:], transpose=True)

                cur = in1
                for li in range(3):
                    lv = levels[li]
                    nxt_lv = levels[li + 1]
                    if li < 2:
                        nxt = inp.tile([128, nxt_lv["w"]], F16, tag=f"in{li + 2}")
                        zero_cols(nxt[:, 0:LEAD])
                        zero_cols(nxt[:, LEAD + nxt_lv["c"]:nxt_lv["w"]])
                    sd_t = sdp.tile([128, 128 * lv["nb"]], F16, tag=f"sd{li + 1}")
                    if 128 * lv["nb"] > lv["s"]:
                        nc.gpsimd.memset(sd_t[:, lv["s"]:128 * lv["nb"]], 0.0)
                    pairs = pairs_view(cur)
                    s0 = 0
                    for ns in chunk_sizes(lv["sp"]):
                        pa_t = pa.tile([128, 512], F32, tag="pa")
                        pd_t = pd.tile([128, 512], F32, tag="pd")
                        conv_chunk(pairs, pa_t[:, 0:ns], "A", s0, ns)
                        conv_chunk(pairs, pd_t[:, 0:ns], "D", s0, ns)
                        nsv = min(ns, lv["s"] - s0)
                        if li < 2:
                            nc.scalar.copy(nxt[:, LEAD + s0:LEAD + s0 + nsv],
                                           pa_t[:, 0:nsv])
                        else:
                            nc.scalar.copy(in4_rows[:, r, LEAD + s0:LEAD + s0 + nsv],
                                           pa_t[:, 0:nsv])
                        nc.scalar.copy(sd_t[:, s0:s0 + nsv], pd_t[:, 0:nsv])
                        s0 += ns
                    out_t = emit_out(sd_t, lv, lv["nb"])
                    dma_out(out_t, d_out[li], r, lv)
                    cur = nxt
                zero_cols(in4[:, r * lv4["w"]:r * lv4["w"] + LEAD])
                zero_cols(in4[:, r * lv4["w"] + LEAD + lv4["c"]:(r + 1) * lv4["w"]])

            # ---------- packed levels 4..6 ----------
            cur = in4
            for li in range(3, LEVEL):
                lv = levels[li]
                last = li == LEVEL - 1
                if not last:
                    nxt_lv = levels[li + 1]
                    nxt = packed.tile([128, rows * nxt_lv["w"]], F16,
                                      tag=f"in{li + 2}")
                    nxt_rows = nxt.rearrange("p (r w) -> p r w", r=rows)
                    for r in range(rows):
                        w_ = nxt_lv["w"]
                        zero_cols(nxt[:, r * w_:r * w_ + LEAD])
                        zero_cols(nxt[:, r * w_ + LEAD + nxt_lv["c"]:(r + 1) * w_])
                blkw = 128 * lv["nb"]
                sd_t = packed.tile([128, rows * blkw], F16, tag=f"sdp{li}")
                sda_t = None
                if last:
                    sda_t = packed.tile([128, rows * blkw], F16, tag="sdpa")
                if blkw > lv["s"]:
                    for r in range(rows):
                        gap = slice(r * blkw + lv["s"], (r + 1) * blkw)
                        nc.gpsimd.memset(sd_t[:, gap], 0.0)
                        if last:
                            nc.gpsimd.memset(sda_t[:, gap], 0.0)
                pairs = pairs_view(cur, rows_packed=rows)
                r0 = 0
                for nr in row_chunks(rows, lv["sp"]):
                    ns_tot = nr * lv["sp"]
                    pa_t = pa.tile([128, 512], F32, tag="pa")
                    pd_t = pd.tile([128, 512], F32, tag="pd")
                    conv_chunk(pairs, pa_t[:, 0:ns_tot], "A", 0, lv["sp"],
                               extra=(r0, nr))
                    conv_chunk(pairs, pd_t[:, 0:ns_tot], "D", 0, lv["sp"],
                               extra=(r0, nr))
                    for i in range(nr):
                        r = r0 + i
                        nc.scalar.copy(
                            sd_t[:, r * blkw:r * blkw + lv["s"]],
                            pd_t[:, i * lv["sp"]:i * lv["sp"] + lv["s"]])
                        sla = pa_t[:, i * lv["sp"]:i * lv["sp"] + lv["s"]]
                        if last:
                            nc.scalar.copy(
                                sda_t[:, r * blkw:r * blkw + lv["s"]], sla)
                        else:
                            nc.scalar.copy(
                                nxt_rows[:, r, LEAD:LEAD + lv["s"]], sla)
                    r0 += nr
                # one batched transpose per packed tile, then per-row stores
                t_t = packed.tile([128, rows * blkw], F16, tag=f"tp{li}")
                nc.scalar.dma_start(
                    out=t_t.rearrange("p (t f) -> p t f", f=128),
                    in_=sd_t[:], transpose=True)
                out_t = packed.tile([128, rows * blkw], F32, tag=f"op{li}")
                nc.vector.tensor_copy(out_t[:], t_t[:])
                if last:
                    ta_t = packed.tile([128, rows * blkw], F16, tag="tpa")
                    nc.scalar.dma_start(
                        out=ta_t.rearrange("p (t f) -> p t f", f=128),
                        in_=sda_t[:], transpose=True)
                    outa_t = packed.tile([128, rows * blkw], F32, tag="opa")
                    nc.vector.tensor_copy(outa_t[:], ta_t[:])
                for r in range(rows):
                    dma_out(out_t, d_out[li], r, lv, base_col=r * blkw)
                    if last:
                        dma_out(outa_t, a6_out, r, lv, base_col=r * blkw)
                if not last:
                    cur = nxt
    nc.compile()
    return nc


_CACHE = {}


def _get_nc():
    if "nc" not in _CACHE:
        _CACHE["nc"] = build_nc()
        _CACHE["w"] = make_band_arrays()
    return _CACHE["nc"], _CACHE["w"]


LAST_RESULT = None


def kernel(x):
    global LAST_RESULT
    x = np.ascontiguousarray(np.asarray(x), dtype=np.float32)
    assert x.shape == (B_FULL, N0)
    from concourse.bass_utils import run_bass_kernel_spmd

    nc, (wmain, whalo) = _get_nc()
    in_maps = [
        {"x": x[c * ROWS:(c + 1) * ROWS], "wmain": wmain, "whalo": whalo}
        for c in range(N_CORES)
    ]
    res = run_bass_kernel_spmd(nc, in_maps, core_ids=list(range(N_CORES)))
    LAST_RESULT = res
    outs = []
    for name in OUT_NAMES:
        outs.append(np.concatenate([res.results[c][name]
                                    for c in range(N_CORES)], axis=0))
    return tuple(outs)
